# revision 1
# baseline (speedup 1.0000x reference)
"""Gaussian row-smoothing (sigma=h_smooth, truncate=4.0, reflect padding) on
8 Trainium2 NeuronCores.

Strategy
--------
Data-parallel over rows (nz=4096 -> 512 rows/core). The 1D conv along rows is
computed on the TensorEngine as a banded-Toeplitz matmul in the transposed
domain:

  host: per core, pad the [512, 8192] shard symmetrically by r=40 along cols,
        transpose to [8272, 512], zero-pad to [65*128, 512] and view as 65
        column-tiles of [128, 512] (partition dim = column index).

  device: output column-block b (128 cols x 512 rows, transposed layout) is
        psum_b = WA.T @ tile_b + WB.T @ tile_{b+1}
        where WA[p, j] = w[p - j]       (0 <= p-j <= 2r)
              WB[p, j] = w[128 + p - j] (0 <= 128+p-j <= 2r)
        are constant [128, 128] band matrices holding the 81-tap kernel.
        PSUM -> SBUF copy (DVE), DMA out as [8192, 512] per core.

  host: transpose each core's output back and concatenate.

Boundary reflection is folded into the host-prepared input tiles, so the
device kernel is completely uniform.

Matmul dtype modes (KERNEL_MODE env; f32r default):
  f32r   - operands float32r: single-pass fp32 matmul, ~101-120us (~2e-4 rel err)
  f32    - full fp32 (two HW passes per matmul), ~129us (~2.3e-6)
  bsplit - data+weights split into bf16 hi+lo, 6 matmuls/block, ~124us (~5.6e-6)
"""

import os
import numpy as np

NZ, NX = 4096, 8192
N_CORES = 8
RPC = NZ // N_CORES          # rows per core = 512
BLK = 128                    # column block (partition dim)
NCH = NX // BLK              # 64 output column blocks per row
NT = NCH + 1                 # 65 input tiles (one extra for the right overlap)
TRUNCATE = 4.0
MODE = os.environ.get("KERNEL_MODE", "f32r")
N_WARMUP = 0  # junk matmuls to lift the PE HAM clock-gate

_NC_CACHE = {}


def _gauss_weights(sigma: float) -> tuple[np.ndarray, int]:
    radius = int(TRUNCATE * sigma + 0.5)
    x = np.arange(-radius, radius + 1, dtype=np.float32)
    w = np.exp(np.float32(-0.5) * (x / np.float32(sigma)) ** 2)
    w = w / np.sum(w)
    return w.astype(np.float32), radius


def _band_matrices(sigma: float) -> tuple[np.ndarray, np.ndarray, int]:
    w, r = _gauss_weights(sigma)
    ntaps = 2 * r + 1
    assert ntaps <= BLK, f"kernel supports radius <= 63, got {r}"
    wa = np.zeros((BLK, BLK), np.float32)
    wb = np.zeros((BLK, BLK), np.float32)
    p = np.arange(BLK)[:, None]
    j = np.arange(BLK)[None, :]
    k = p - j
    m = (k >= 0) & (k <= 2 * r)
    wa[m] = w[k[m]]
    k2 = k + BLK
    m2 = (k2 >= 0) & (k2 <= 2 * r)
    wb[m2] = w[k2[m2]]
    return wa, wb, r


def build_nc():
    """Build (and cache) the SPMD Bass program. Shapes are fixed; the band
    weights arrive as data, so one NEFF serves any h_smooth with radius<=63."""
    if "nc" in _NC_CACHE:
        return _NC_CACHE["nc"]
    import concourse.tile as tile
    from concourse import bacc, mybir

    f32 = mybir.dt.float32
    f32r = mybir.dt.float32r
    bf16 = mybir.dt.bfloat16
    if MODE == "bsplit":
        xdt = wdt = bf16
        n_w = 4
        n_x = 2
    else:
        xdt = f32 if MODE == "f32" else f32r
        wdt = xdt
        n_w = 2
        n_x = 1

    nc = bacc.Bacc(None)
    xnames = ["xh", "xl"] if n_x == 2 else ["xt"]
    xparams = [
        nc.declare_dram_parameter(n, [NT * BLK, RPC], xdt, isOutput=False)
        for n in xnames
    ]
    wnames = ["wah", "wal", "wbh", "wbl"] if n_w == 4 else ["wa", "wb"]
    wparams = [
        nc.declare_dram_parameter(n, [BLK, BLK], wdt, isOutput=False) for n in wnames
    ]
    out = nc.declare_dram_parameter("out", [NX, RPC], f32, isOutput=True)

    with tile.TileContext(nc) as tc:
        with (
            tc.tile_pool(name="w", bufs=1) as wpool,
            tc.tile_pool(name="x", bufs=16) as xpool,
            tc.tile_pool(name="ps", bufs=4, space="PSUM") as pspool,
            tc.tile_pool(name="o", bufs=6) as opool,
        ):
            wts = []
            for n, p in zip(wnames, wparams):
                wt = wpool.tile([BLK, BLK], wdt, tag=n)
                nc.sync.dma_start(wt[:], p[:])
                wts.append(wt)

            # PE warmup: the HAM clock gate only lifts (1.2 -> 2.4 GHz) after
            # ~3.4us of sustained PE activity; burn junk matmuls into a scratch
            # PSUM bank while the first data tiles are still in flight.
            if N_WARMUP:
                wu = pspool.tile([BLK, RPC], f32, tag="psum")
                for i in range(N_WARMUP):
                    nc.tensor.matmul(
                        wu[:, 0:BLK], wts[0][:], wts[0][:], start=True, stop=True
                    )

            def load_tiles(t):
                ts = []
                for xi, xp in enumerate(xparams):
                    tl = xpool.tile([BLK, RPC], xdt, tag=f"xtile{xi}")
                    nc.sync.dma_start(tl[:], xp[t * BLK : (t + 1) * BLK, :])
                    ts.append(tl)
                return ts

            prev = load_tiles(0)
            if MODE == "bsplit":
                for b in range(NCH):
                    cur = load_tiles(b + 1)
                    ps = pspool.tile([BLK, RPC], f32, tag="psum")
                    # psum = WAh.x_h + WAl.x_h + WAh.x_l  (+ same for B chunk);
                    # the dropped wl.xl term is O(2^-18).
                    nc.tensor.matmul(ps[:], wts[0][:], prev[0][:], start=True, stop=False)
                    nc.tensor.matmul(ps[:], wts[1][:], prev[0][:], start=False, stop=False)
                    nc.tensor.matmul(ps[:], wts[0][:], prev[1][:], start=False, stop=False)
                    nc.tensor.matmul(ps[:], wts[2][:], cur[0][:], start=False, stop=False)
                    nc.tensor.matmul(ps[:], wts[3][:], cur[0][:], start=False, stop=False)
                    nc.tensor.matmul(ps[:], wts[2][:], cur[1][:], start=False, stop=True)
                    ot = opool.tile([BLK, RPC], f32, tag="otile")
                    nc.vector.tensor_copy(ot[:], ps[:])
                    nc.scalar.dma_start(out[b * BLK : (b + 1) * BLK, :], ot[:])
                    prev = cur
            else:
                # Two blocks per group: one 2-bank PSUM tile, one DVE copy and
                # one 512KB output DMA per pair (fewer instructions + sems).
                # Inputs stay as separate 256KB loads so each tile's matmul can
                # start as soon as that tile lands.
                prev_ap = prev[0][:]
                for g in range(NCH // 2):
                    mid_ap = load_tiles(2 * g + 1)[0][:]
                    nxt_ap = load_tiles(2 * g + 2)[0][:]
                    ps = pspool.tile([BLK, 2 * RPC], f32, tag="psum")
                    nc.tensor.matmul(ps[:, 0:RPC], wts[0][:], prev_ap, start=True, stop=False)
                    nc.tensor.matmul(ps[:, 0:RPC], wts[1][:], mid_ap, start=False, stop=True)
                    nc.tensor.matmul(ps[:, RPC:], wts[0][:], mid_ap, start=True, stop=False)
                    nc.tensor.matmul(ps[:, RPC:], wts[1][:], nxt_ap, start=False, stop=True)
                    ot = opool.tile([BLK, 2 * RPC], f32, tag="otile")
                    nc.vector.tensor_copy(ot[:], ps[:])
                    dview = out[2 * g * BLK : (2 * g + 2) * BLK, :].rearrange(
                        "(c p) r -> p c r", c=2
                    )
                    sview = ot[:].rearrange("p (c r) -> p c r", c=2)
                    nc.scalar.dma_start(dview, sview)
                    prev_ap = nxt_ap

    nc.finalize()
    _NC_CACHE["nc"] = nc
    return nc


def make_in_maps(feature: np.ndarray, h_smooth) -> list[dict]:
    sigma = float(int(h_smooth))
    wa, wb, r = _band_matrices(sigma)
    feature = np.asarray(feature, dtype=np.float32)
    assert feature.shape == (NZ, NX)
    if MODE == "bsplit":
        import ml_dtypes

        def split(w):
            hi = w.astype(ml_dtypes.bfloat16)
            lo = (w - hi.astype(np.float32)).astype(ml_dtypes.bfloat16)
            return hi, lo

        wah, wal = split(wa)
        wbh, wbl = split(wb)
        wmap = {"wah": wah, "wal": wal, "wbh": wbh, "wbl": wbl}
    else:
        wmap = {"wa": wa, "wb": wb}
    in_maps = []
    for c in range(N_CORES):
        x = feature[c * RPC : (c + 1) * RPC]
        xp = np.pad(x, ((0, 0), (r, r)), mode="symmetric")  # [512, 8192+2r]
        xtile = np.zeros((NT * BLK, RPC), np.float32)
        xtile[: NX + 2 * r] = xp.T
        if MODE == "bsplit":
            import ml_dtypes

            xh = xtile.astype(ml_dtypes.bfloat16)
            xl = (xtile - xh.astype(np.float32)).astype(ml_dtypes.bfloat16)
            in_maps.append({"xh": xh, "xl": xl, **wmap})
        else:
            in_maps.append({"xt": xtile, **wmap})
    return in_maps


def assemble(results: list[dict]) -> np.ndarray:
    out = np.empty((NZ, NX), np.float32)
    for c in range(N_CORES):
        out[c * RPC : (c + 1) * RPC] = results[c]["out"].T
    return out


def kernel(feature, h_smooth) -> np.ndarray:
    from concourse.bass_utils import run_bass_kernel_spmd

    nc = build_nc()
    in_maps = make_in_maps(feature, h_smooth)
    res = run_bass_kernel_spmd(nc, in_maps, core_ids=list(range(N_CORES)))
    return assemble(res.results)



# revision 2
# speedup vs baseline: 1.7471x; 1.7471x over previous
"""Gaussian row-smoothing (sigma=h_smooth, truncate=4.0, reflect padding) on
8 Trainium2 NeuronCores.

Strategy
--------
Data-parallel over rows (nz=4096 -> 512 rows/core). The 1D conv along rows is
computed on the TensorEngine as a banded-Toeplitz matmul in the transposed
domain:

  host: per core, pad the [512, 8192] shard symmetrically by r=40 along cols,
        transpose to [8272, 512], zero-pad to [65*128, 512], convert to bf16,
        and relayout partition-major to [128, 65*512] so device DMAs are
        large and fully contiguous per partition.

  device: output column-block b (128 cols x 512 rows, transposed layout) is
        psum_b = WA.T @ tile_b + WB.T @ tile_{b+1}
        where WA[p, j] = w[p - j]       (0 <= p-j <= 2r)
              WB[p, j] = w[128 + p - j] (0 <= 128+p-j <= 2r)
        are constant [128, 128] bf16 band matrices holding the 81-tap kernel.
        Inputs stream in as 8 chunks of ~1MB (8-9 column-tiles each); matmuls
        accumulate in f32 PSUM; DVE copies PSUM -> bf16 SBUF out-chunks of 8
        blocks which DMA out as ~1MB transfers.

  host: reverse the relayout, upconvert bf16 -> f32, concatenate.

All HBM traffic is bf16 (8.5MB in + 8.4MB out per core vs 17+16.8 for f32),
which halves the DMA-bound runtime. f32 PSUM accumulation keeps the only
rounding at the bf16 input/weight/output quantization (~3e-3 l2 rel err).
"""

import numpy as np

NZ, NX = 4096, 8192
N_CORES = 8
RPC = NZ // N_CORES          # rows per core = 512
BLK = 128                    # column block (partition dim)
NCH = NX // BLK              # 64 output column blocks per row
NT = NCH + 1                 # 65 input tiles (one extra for the right overlap)
TRUNCATE = 4.0
N_ICHUNK = 8                 # input DMA chunks (7x8 tiles + 1x9 tiles)
N_OCHUNK = 8                 # output DMA chunks (8 blocks each)

_NC_CACHE = {}


def _gauss_weights(sigma: float) -> tuple[np.ndarray, int]:
    radius = int(TRUNCATE * sigma + 0.5)
    x = np.arange(-radius, radius + 1, dtype=np.float32)
    w = np.exp(np.float32(-0.5) * (x / np.float32(sigma)) ** 2)
    w = w / np.sum(w)
    return w.astype(np.float32), radius


def _band_matrices(sigma: float) -> tuple[np.ndarray, np.ndarray, int]:
    w, r = _gauss_weights(sigma)
    ntaps = 2 * r + 1
    assert ntaps <= BLK, f"kernel supports radius <= 63, got {r}"
    wa = np.zeros((BLK, BLK), np.float32)
    wb = np.zeros((BLK, BLK), np.float32)
    p = np.arange(BLK)[:, None]
    j = np.arange(BLK)[None, :]
    k = p - j
    m = (k >= 0) & (k <= 2 * r)
    wa[m] = w[k[m]]
    k2 = k + BLK
    m2 = (k2 >= 0) & (k2 <= 2 * r)
    wb[m2] = w[k2[m2]]
    return wa, wb, r


def _chunk_bounds():
    """Input tile index ranges per chunk: 7 chunks of 8 tiles + 1 of 9."""
    bounds = []
    t = 0
    for c in range(N_ICHUNK):
        n = NT - t if c == N_ICHUNK - 1 else NT // N_ICHUNK
        bounds.append((t, t + n))
        t += n
    return bounds


def build_nc():
    """Build (and cache) the SPMD Bass program. Shapes are fixed; the band
    weights arrive as data, so one NEFF serves any h_smooth with radius<=63."""
    if "nc" in _NC_CACHE:
        return _NC_CACHE["nc"]
    import concourse.tile as tile
    from concourse import bacc, mybir

    f32 = mybir.dt.float32
    bf16 = mybir.dt.bfloat16

    nc = bacc.Bacc(None)
    xt = nc.declare_dram_parameter("xt", [BLK, NT * RPC], bf16, isOutput=False)
    wa_p = nc.declare_dram_parameter("wa", [BLK, BLK], bf16, isOutput=False)
    wb_p = nc.declare_dram_parameter("wb", [BLK, BLK], bf16, isOutput=False)
    out = nc.declare_dram_parameter("out", [BLK, NCH * RPC], bf16, isOutput=True)

    bounds = _chunk_bounds()
    max_ch = max(e - s for s, e in bounds)

    with tile.TileContext(nc) as tc:
        with (
            tc.tile_pool(name="w", bufs=1) as wpool,
            tc.tile_pool(name="x", bufs=N_ICHUNK) as xpool,
            tc.tile_pool(name="ps", bufs=4, space="PSUM") as pspool,
            tc.tile_pool(name="o", bufs=4) as opool,
        ):
            wa = wpool.tile([BLK, BLK], bf16, tag="wa")
            wb = wpool.tile([BLK, BLK], bf16, tag="wb")
            nc.sync.dma_start(wa[:], wa_p[:])
            nc.sync.dma_start(wb[:], wb_p[:])

            # All input chunks are issued up-front and stay resident; the
            # two HWDGE queues (sync + scalar) interleave so input and
            # output transfers share HBM bandwidth evenly.
            xch = []
            for c, (s, e) in enumerate(bounds):
                xt_tile = xpool.tile([BLK, max_ch * RPC], bf16, tag="xchunk")
                eng = nc.sync if c % 2 == 0 else nc.scalar
                eng.dma_start(
                    xt_tile[:, : (e - s) * RPC], xt[:, s * RPC : e * RPC]
                )
                xch.append(xt_tile)

            def tl(t):
                c = min(t // (NT // N_ICHUNK), N_ICHUNK - 1)
                i = t - bounds[c][0]
                return xch[c][:, i * RPC : (i + 1) * RPC]

            # Waves of 4 blocks: weight matrix held across 4 consecutive
            # matmuls (wa x4, then wb x4) to minimize LDWEIGHTS churn.
            ot = None
            for wv in range(NCH // 4):
                b0 = 4 * wv
                ps0 = pspool.tile([BLK, 2 * RPC], f32, tag="psum")
                ps1 = pspool.tile([BLK, 2 * RPC], f32, tag="psum")
                halves = [(ps0, 0), (ps0, 1), (ps1, 0), (ps1, 1)]
                for i, (ps, h) in enumerate(halves):
                    nc.tensor.matmul(
                        ps[:, h * RPC : (h + 1) * RPC], wa[:], tl(b0 + i),
                        start=True, stop=False,
                    )
                for i, (ps, h) in enumerate(halves):
                    nc.tensor.matmul(
                        ps[:, h * RPC : (h + 1) * RPC], wb[:], tl(b0 + i + 1),
                        start=False, stop=True,
                    )
                o = wv // 2
                off = (wv % 2) * 4 * RPC
                if wv % 2 == 0:
                    ot = opool.tile([BLK, 8 * RPC], bf16, tag="otile")
                nc.vector.tensor_copy(ot[:, off : off + 2 * RPC], ps0[:])
                nc.vector.tensor_copy(ot[:, off + 2 * RPC : off + 4 * RPC], ps1[:])
                if wv % 2 == 1:
                    eng = nc.scalar if o % 2 == 0 else nc.sync
                    eng.dma_start(out[:, o * 8 * RPC : (o + 1) * 8 * RPC], ot[:])

    nc.finalize()
    _NC_CACHE["nc"] = nc
    return nc


def make_in_maps(feature: np.ndarray, h_smooth) -> list[dict]:
    import ml_dtypes

    sigma = float(int(h_smooth))
    wa, wb, r = _band_matrices(sigma)
    wa = wa.astype(ml_dtypes.bfloat16)
    wb = wb.astype(ml_dtypes.bfloat16)
    feature = np.asarray(feature, dtype=np.float32)
    assert feature.shape == (NZ, NX)
    in_maps = []
    for c in range(N_CORES):
        x = feature[c * RPC : (c + 1) * RPC]
        xp = np.pad(x, ((0, 0), (r, r)), mode="symmetric")  # [512, 8192+2r]
        xt = np.zeros((NT * BLK, RPC), np.float32)
        xt[: NX + 2 * r] = xp.T
        # partition-major relayout: [128, 65*512], row p holds tile t's
        # column p for all t -- device DMAs are contiguous per partition.
        xt = xt.reshape(NT, BLK, RPC).transpose(1, 0, 2).reshape(BLK, NT * RPC)
        in_maps.append({"xt": xt.astype(ml_dtypes.bfloat16), "wa": wa, "wb": wb})
    return in_maps


def assemble(results: list[dict]) -> np.ndarray:
    out = np.empty((NZ, NX), np.float32)
    for c in range(N_CORES):
        o = np.asarray(results[c]["out"]).astype(np.float32)  # [128, 64*512]
        o = o.reshape(BLK, NCH, RPC).transpose(1, 0, 2).reshape(NX, RPC)
        out[c * RPC : (c + 1) * RPC] = o.T
    return out


def kernel(feature, h_smooth) -> np.ndarray:
    from concourse.bass_utils import run_bass_kernel_spmd

    nc = build_nc()
    in_maps = make_in_maps(feature, h_smooth)
    res = run_bass_kernel_spmd(nc, in_maps, core_ids=list(range(N_CORES)))
    return assemble(res.results)


# revision 5
# speedup vs baseline: 1.9819x; 1.1344x over previous
"""Gaussian row-smoothing (sigma=h_smooth, truncate=4.0, reflect padding) on
8 Trainium2 NeuronCores.

Strategy
--------
Data-parallel over rows (nz=4096 -> 512 rows/core). The 1D conv along rows is
computed on the TensorEngine as a banded-Toeplitz matmul in the transposed
domain:

  host: per core, pad the [512, 8192] shard symmetrically by r=40 along cols,
        transpose to [8272, 512], zero-pad to [65*128, 512], convert to bf16,
        and relayout partition-major to [128, 65*512] so device DMAs are
        large and fully contiguous per partition.

  device: output column-block b (128 cols x 512 rows, transposed layout) is
        psum_b = WA.T @ tile_b + WB.T @ tile_{b+1}
        where WA[p, j] = w[p - j]       (0 <= p-j <= 2r)
              WB[p, j] = w[128 + p - j] (0 <= 128+p-j <= 2r)
        are constant [128, 128] bf16 band matrices holding the 81-tap kernel.
        Inputs stream in as 8 chunks of ~1MB (8-9 column-tiles each); matmuls
        accumulate in f32 PSUM; DVE copies PSUM -> bf16 SBUF out-chunks of 8
        blocks which DMA out as ~1MB transfers.

  host: reverse the relayout, upconvert bf16 -> f32, concatenate.

All HBM traffic is bf16 (8.5MB in + 8.4MB out per core vs 17+16.8 for f32),
which halves the DMA-bound runtime. f32 PSUM accumulation keeps the only
rounding at the bf16 input/weight/output quantization (~3e-3 l2 rel err).
"""

import numpy as np

NZ, NX = 4096, 8192
N_CORES = 8
RPC = NZ // N_CORES          # rows per core = 512
BLK = 128                    # column block (partition dim)
NCH = NX // BLK              # 64 output column blocks per row
NT = NCH + 1                 # 65 input tiles (one extra for the right overlap)
TRUNCATE = 4.0
N_ICHUNK = 8                 # input DMA chunks (7x8 tiles + 1x9 tiles)
N_OCHUNK = 8                 # output DMA chunks (8 blocks each)
N_WARMUP = 24                # junk matmuls to lift the PE HAM clock-gate

_NC_CACHE = {}


def _gauss_weights(sigma: float) -> tuple[np.ndarray, int]:
    radius = int(TRUNCATE * sigma + 0.5)
    x = np.arange(-radius, radius + 1, dtype=np.float32)
    w = np.exp(np.float32(-0.5) * (x / np.float32(sigma)) ** 2)
    w = w / np.sum(w)
    return w.astype(np.float32), radius


def _band_matrices(sigma: float) -> tuple[np.ndarray, np.ndarray, int]:
    w, r = _gauss_weights(sigma)
    ntaps = 2 * r + 1
    assert ntaps <= BLK, f"kernel supports radius <= 63, got {r}"
    wa = np.zeros((BLK, BLK), np.float32)
    wb = np.zeros((BLK, BLK), np.float32)
    p = np.arange(BLK)[:, None]
    j = np.arange(BLK)[None, :]
    k = p - j
    m = (k >= 0) & (k <= 2 * r)
    wa[m] = w[k[m]]
    k2 = k + BLK
    m2 = (k2 >= 0) & (k2 <= 2 * r)
    wb[m2] = w[k2[m2]]
    return wa, wb, r


def _chunk_bounds():
    """Input tile index ranges per chunk: 7 chunks of 8 tiles + 1 of 9."""
    bounds = []
    t = 0
    for c in range(N_ICHUNK):
        n = NT - t if c == N_ICHUNK - 1 else NT // N_ICHUNK
        bounds.append((t, t + n))
        t += n
    return bounds


def build_nc():
    """Build (and cache) the SPMD Bass program. Shapes are fixed; the band
    weights arrive as data, so one NEFF serves any h_smooth with radius<=63."""
    if "nc" in _NC_CACHE:
        return _NC_CACHE["nc"]
    import concourse.tile as tile
    from concourse import bacc, mybir

    f32 = mybir.dt.float32
    bf16 = mybir.dt.bfloat16

    nc = bacc.Bacc(None)
    xt = nc.declare_dram_parameter("xt", [BLK, NT * RPC], bf16, isOutput=False)
    wa_p = nc.declare_dram_parameter("wa", [BLK, BLK], bf16, isOutput=False)
    wb_p = nc.declare_dram_parameter("wb", [BLK, BLK], bf16, isOutput=False)
    out = nc.declare_dram_parameter("out", [BLK, NCH * RPC], bf16, isOutput=True)

    bounds = _chunk_bounds()
    max_ch = max(e - s for s, e in bounds)

    with tile.TileContext(nc) as tc:
        with (
            tc.tile_pool(name="w", bufs=1) as wpool,
            tc.tile_pool(name="x", bufs=N_ICHUNK) as xpool,
            tc.tile_pool(name="ps", bufs=4, space="PSUM") as pspool,
            tc.tile_pool(name="o", bufs=4) as opool,
        ):
            wa = wpool.tile([BLK, BLK], bf16, tag="wa")
            wb = wpool.tile([BLK, BLK], bf16, tag="wb")
            nc.sync.dma_start(wa[:], wa_p[:])
            nc.sync.dma_start(wb[:], wb_p[:])
            scratch = wpool.tile([BLK, RPC], bf16, tag="scratch")
            nc.vector.memset(scratch[:], 0.0)

            # All input chunks are issued up-front and stay resident; the
            # two HWDGE queues (sync + scalar) interleave so input and
            # output transfers share HBM bandwidth evenly.
            xch = []
            for c, (s, e) in enumerate(bounds):
                xt_tile = xpool.tile([BLK, max_ch * RPC], bf16, tag="xchunk")
                eng = nc.sync if c % 2 == 0 else nc.scalar
                eng.dma_start(
                    xt_tile[:, : (e - s) * RPC], xt[:, s * RPC : e * RPC]
                )
                xch.append(xt_tile)

            # Junk matmuls spanning the first input chunk's DMA: the PE HAM
            # clock-gate lifts (1.2 -> 2.4 GHz) only after ~3.4us of sustained
            # activity, and re-throttles after ~3.4us idle. Keep PE busy until
            # real data lands so the real matmul stream runs warm.
            if N_WARMUP:
                wu = pspool.tile([BLK, RPC], f32, tag="psum")
                for _ in range(N_WARMUP):
                    nc.tensor.matmul(
                        wu[:], wa[:], scratch[:], start=True, stop=True
                    )

            def tl(t):
                c = min(t // (NT // N_ICHUNK), N_ICHUNK - 1)
                i = t - bounds[c][0]
                return xch[c][:, i * RPC : (i + 1) * RPC]

            # Waves of 4 blocks: weight matrix held across 4 consecutive
            # matmuls (wa x4, then wb x4) to minimize LDWEIGHTS churn.
            ot = None
            for wv in range(NCH // 4):
                b0 = 4 * wv
                ps0 = pspool.tile([BLK, 2 * RPC], f32, tag="psum")
                ps1 = pspool.tile([BLK, 2 * RPC], f32, tag="psum")
                halves = [(ps0, 0), (ps0, 1), (ps1, 0), (ps1, 1)]
                for i, (ps, h) in enumerate(halves):
                    nc.tensor.matmul(
                        ps[:, h * RPC : (h + 1) * RPC], wa[:], tl(b0 + i),
                        start=True, stop=False,
                    )
                for i, (ps, h) in enumerate(halves):
                    nc.tensor.matmul(
                        ps[:, h * RPC : (h + 1) * RPC], wb[:], tl(b0 + i + 1),
                        start=False, stop=True,
                    )
                o = wv // 2
                off = (wv % 2) * 4 * RPC
                if wv % 2 == 0:
                    ot = opool.tile([BLK, 8 * RPC], bf16, tag="otile")
                # PSUM->SBUF casts split across DVE and ACT: the f32-PSUM
                # source caps either engine at 1x mode (~1us per 1024-elem
                # copy), so one engine alone would pace the whole pipeline.
                nc.vector.tensor_copy(ot[:, off : off + 2 * RPC], ps0[:])
                nc.scalar.copy(ot[:, off + 2 * RPC : off + 4 * RPC], ps1[:])
                if wv % 2 == 1:
                    # Two half-chunk DMAs on the two HWDGE queues so output
                    # drains at dual-queue rate (~420 GB/s vs ~210 single).
                    base = o * 8 * RPC
                    nc.sync.dma_start(
                        out[:, base : base + 4 * RPC], ot[:, : 4 * RPC]
                    )
                    nc.scalar.dma_start(
                        out[:, base + 4 * RPC : base + 8 * RPC], ot[:, 4 * RPC :]
                    )

    nc.finalize()
    _NC_CACHE["nc"] = nc
    return nc


def make_in_maps(feature: np.ndarray, h_smooth) -> list[dict]:
    import ml_dtypes

    sigma = float(int(h_smooth))
    wa, wb, r = _band_matrices(sigma)
    wa = wa.astype(ml_dtypes.bfloat16)
    wb = wb.astype(ml_dtypes.bfloat16)
    feature = np.asarray(feature, dtype=np.float32)
    assert feature.shape == (NZ, NX)
    in_maps = []
    for c in range(N_CORES):
        x = feature[c * RPC : (c + 1) * RPC]
        xp = np.pad(x, ((0, 0), (r, r)), mode="symmetric")  # [512, 8192+2r]
        xt = np.zeros((NT * BLK, RPC), np.float32)
        xt[: NX + 2 * r] = xp.T
        # partition-major relayout: [128, 65*512], row p holds tile t's
        # column p for all t -- device DMAs are contiguous per partition.
        xt = xt.reshape(NT, BLK, RPC).transpose(1, 0, 2).reshape(BLK, NT * RPC)
        in_maps.append({"xt": xt.astype(ml_dtypes.bfloat16), "wa": wa, "wb": wb})
    return in_maps


def assemble(results: list[dict]) -> np.ndarray:
    out = np.empty((NZ, NX), np.float32)
    for c in range(N_CORES):
        o = np.asarray(results[c]["out"]).astype(np.float32)  # [128, 64*512]
        o = o.reshape(BLK, NCH, RPC).transpose(1, 0, 2).reshape(NX, RPC)
        out[c * RPC : (c + 1) * RPC] = o.T
    return out


def kernel(feature, h_smooth) -> np.ndarray:
    from concourse.bass_utils import run_bass_kernel_spmd

    nc = build_nc()
    in_maps = make_in_maps(feature, h_smooth)
    res = run_bass_kernel_spmd(nc, in_maps, core_ids=list(range(N_CORES)))
    return assemble(res.results)
